# revision 1
# baseline (speedup 1.0000x reference)
"""KNN top-5 kernel for Trainium2 (Bass/Tile), SPMD over 8 NeuronCores.

Problem: x [16384, 256] f32, reference_points [100, 256] f32.
Output: indices [16384, 5] int32 of the 5 nearest reference points per row
(ascending distance, ties -> lower index), matching
jax.lax.top_k(-||x - r||, 5).

Strategy:
  - Data parallel: 2048 rows of x per core; reference table replicated.
  - Ranking by v = 2*x.r - ||r||^2 = ||x||^2 - d^2 (per-row monotone in -d),
    computed on the PE: PSUM[128,100] = ones^T@(-rn2) + xT_k0^T@(2 refT_k0)
    + xT_k1^T@(2 refT_k1).  x is passed host-transposed so the contraction
    dim (d) is the partition dim with no on-chip transposes.
  - Top-5: DVE max (top-8 values desc) + max_index (their indices; ties get
    ascending indices, matching top_k tie-breaking).
  - PE Matmult instructions only support a single sync-wait, so every matmul
    input is covered by one DMA: consts are packed into one [128, 428]
    tensor/DMA, and both K-halves of each x^T chunk ship in one 3D-AP DMA.
"""

import numpy as np

import concourse.bass as bass  # noqa: F401  (AP helpers)
import concourse.mybir as mybir
from concourse import bacc, tile
from concourse.bass_utils import run_bass_kernel_spmd

N_CORES = 8
B = 16384          # total rows
D = 256            # feature dim
P = 100            # number of reference points
ROWS_PER_CORE = B // N_CORES      # 2048
ROW_TILE = 128
N_ROW_TILES = ROWS_PER_CORE // ROW_TILE   # 16
# x^T ships in 3 serialized SWDGE chunks (row-tile counts 6/5/5).  The drain
# at kernel tail supports at most 8 sync waits = one per sem domain, which
# caps (#SW DMA lanes + #HW DMA lanes + #engines): 3 + 2 + 3 here.
CHUNK_TILES = [6, 5, 5]

# consts layout (one [128, CONST_W] f32 tensor):
#   [:, 0:100]    refq0  (2*r^T rows 0..127)
#   [:, 100:200]  refq1  (2*r^T rows 128..255)
#   [0, 200:328]  ones   (K=1 lhsT for the bias matmul)
#   [0, 328:428]  -||r||^2
CONST_W = 428

_cached = {}


def _build_bass():
    # Bacc (not plain Bass): its compile() runs move_matmul_waits_to_ldweights
    # + generate_event_semaphores, which split multi-sem waits to satisfy the
    # 1-wait-per-instruction hardware limit.
    nc = bacc.Bacc("TRN2")

    xt = nc.dram_tensor("xt", [D, ROWS_PER_CORE], mybir.dt.float32,
                        kind="ExternalInput")
    consts = nc.dram_tensor("consts", [128, CONST_W], mybir.dt.float32,
                            kind="ExternalInput")
    out_idx = nc.dram_tensor("out_idx", [ROWS_PER_CORE, 8], mybir.dt.uint32,
                             kind="ExternalOutput")

    # view with the two K-halves split out: xtv[p, a, n] = xt[a*128 + p, n]
    xtv = xt.rearrange("(a p) n -> p a n", a=2)

    with tile.TileContext(nc) as tc:
        with (
            tc.tile_pool(name="consts", bufs=1) as cpool,
            tc.tile_pool(name="xt", bufs=1) as xpool,
            tc.tile_pool(name="dist", bufs=N_ROW_TILES) as spool,
            tc.tile_pool(name="top", bufs=N_ROW_TILES) as tpool,
            tc.tile_pool(name="psum", bufs=8, space="PSUM") as ppool,
        ):
            consts_t = cpool.tile([128, CONST_W], mybir.dt.float32)
            nc.sync.dma_start(consts_t[:], consts[:])
            refq_t = [consts_t[:, 0:P], consts_t[:, P:2 * P]]
            ones_t = consts_t[0:1, 200:200 + ROW_TILE]
            rn2m_t = consts_t[0:1, 328:328 + P]

            # SWDGE has a single physical descriptor ring, so these chunks
            # drain strictly in order -> chunk j's data (and sem) lands at
            # ~proportional time, letting compute pipeline behind the stream.
            xt_t = []
            col = 0
            for j, ntiles in enumerate(CHUNK_TILES):
                w = ntiles * ROW_TILE
                t = xpool.tile([128, 2, w], mybir.dt.float32, name=f"xt_{j}")
                nc.gpsimd.dma_start(t[:], xtv[:, :, col:col + w])
                xt_t.append((t, col))
                col += w

            # all 16 row-tiles' index results accumulate here; one DMA out
            stage = tpool.tile([128, N_ROW_TILES * 8], mybir.dt.uint32,
                               name="stage", tag="stage")

            tile_chunk = []    # row-tile index -> (chunk tile, col offset)
            for (t, col), ntiles in zip(xt_t, CHUNK_TILES):
                for k in range(ntiles):
                    tile_chunk.append((t, k * ROW_TILE))

            for i in range(N_ROW_TILES):
                xt_tile, c = tile_chunk[i]
                p = ppool.tile([ROW_TILE, P], mybir.dt.float32,
                               name=f"psum_{i}", tag="psum")
                # PSUM = ones^T @ (-||r||^2)  (broadcast bias)
                nc.tensor.matmul(p[:], ones_t, rn2m_t,
                                 start=True, stop=False)
                # PSUM += x_chunk^T @ (2 r^T), both K-halves
                nc.tensor.matmul(p[:], xt_tile[:, 0, c:c + ROW_TILE],
                                 refq_t[0], start=False, stop=False)
                nc.tensor.matmul(p[:], xt_tile[:, 1, c:c + ROW_TILE],
                                 refq_t[1], start=False, stop=True)

                s = spool.tile([ROW_TILE, P], mybir.dt.float32,
                               name=f"s_{i}", tag="s")
                nc.scalar.copy(s[:], p[:])

                v8 = tpool.tile([ROW_TILE, 8], mybir.dt.float32,
                                name=f"v8_{i}", tag="v8")
                nc.vector.max(out=v8[:], in_=s[:])
                nc.vector.max_index(out=stage[:, i * 8:(i + 1) * 8],
                                    in_max=v8[:], in_values=s[:])

            # out_idx[t*128 + p, k] = stage[p, t*8 + k]
            stage_v = stage[:].rearrange("p (t k) -> p t k", k=8)
            out_v = out_idx.rearrange("(t p) k -> p t k", p=ROW_TILE)
            nc.sync.dma_start(out_v, stage_v)

    nc.compile()
    return nc


def _make_consts(r: np.ndarray) -> np.ndarray:
    refq = (2.0 * r.T.astype(np.float64)).astype(np.float32)   # [256, 100]
    rn2m = (-(r.astype(np.float64) ** 2).sum(axis=1)).astype(np.float32)
    consts = np.zeros((128, CONST_W), dtype=np.float32)
    consts[:, 0:P] = refq[0:128]
    consts[:, P:2 * P] = refq[128:256]
    consts[0, 200:200 + ROW_TILE] = 1.0
    consts[0, 328:328 + P] = rn2m
    return consts


def kernel(x: np.ndarray, reference_points: np.ndarray) -> np.ndarray:
    assert x.shape == (B, D) and reference_points.shape == (P, D)
    x = np.asarray(x, dtype=np.float32)
    r = np.asarray(reference_points, dtype=np.float32)

    xt = np.ascontiguousarray(x.T)                      # [256, 16384]
    consts = _make_consts(r)

    if "nc" not in _cached:
        _cached["nc"] = _build_bass()
    nc = _cached["nc"]

    in_maps = []
    for c in range(N_CORES):
        slab = np.ascontiguousarray(
            xt[:, c * ROWS_PER_CORE:(c + 1) * ROWS_PER_CORE])
        in_maps.append({"xt": slab, "consts": consts})

    res = run_bass_kernel_spmd(nc, in_maps, core_ids=list(range(N_CORES)))
    _cached["last_result"] = res  # exec_time_ns etc. when BASS_TRACE=1

    out = np.concatenate(
        [res.results[c]["out_idx"][:, :5] for c in range(N_CORES)], axis=0)
    return out.astype(np.int32)

